# revision 40
# baseline (speedup 1.0000x reference)
"""HGATConv (4-head graph attention, N=4096, F=512) on 8 Trainium2 NeuronCores.

Sharding: (head, node-half) grid — core c handles head c//2 and output rows
q*2048..(q+1)*2048 (q = c%2). Every core computes its head's h = x @ W_h for
ALL nodes locally (bf16 matmuls) — no collective at all. The host rotates
the node axis per core so each core's own nodes come first (attention sums
are j-order invariant); that makes the per-core x layout identical SPMD-wise.

Attention math per 128-node j-block (tiles are [128 j, 2048 i]): all scores
are divided by exp(s1_i), which is constant per softmax row and cancels in
the normalization:
  p'' = exp(leakyrelu(s1_i + s2_j)) / exp(s1_i)
      = max(exp(-0.8 s1_i + 0.2 s2_j), exp(s2_j))
      = max(E8_i * g_j, e1_j)          (separable rank-1 first branch!)
with E8_i = exp(-0.8 s1_i) a jb-invariant broadcast tile, g_j = exp(0.2 s2_j)
and e1_j = exp(s2_j) per-partition scalars. So the whole score+leakyrelu is
ONE 4x-rate two-scalar tensor_scalar per block — no wide ACT exp at all; the
only wide 2x op left is the adjacency-mask multiply.
Row sums ride along as a ones-column appended to h (129-wide matmuls).

Mask DMAs issue from the (otherwise idle) GpSimd queue so the Sync queue's
serial dma_start issues don't gate the stream; x arrives in four
k-interleaved quarter DMAs so compute starts after the first MiB.
"""

import sys
import numpy as np

if "/opt/trn_rl_repo" not in sys.path:
    sys.path.insert(0, "/opt/trn_rl_repo")

H, D = 4, 128          # heads, head dim
N, F = 4096, 512       # nodes, features
M = 8                  # cores
NOWN = 1024 * 2        # 2048 own output rows per core
JB = N // 128          # 32 j blocks
IB = NOWN // 128       # 16 own-row blocks
KB = F // 128          # 4 contraction blocks
DA = D + 2             # head W columns + wa2 + wa1
NQ = N // 4            # nodes per x-quarter DMA
LAG = 4                # h-compute blocks emitted ahead of attention blocks

_CACHE = {}

# attention-matmul emission order: s=0 slices first (their start=True clears
# the bank), consecutive matmuls on different PSUM banks
_MM_ORDER = [0, 3, 6, 9, 12, 15, 1, 4, 7, 10, 13, 2, 5, 8, 11, 14]


def _build_nc():
    import concourse.bacc as bacc
    from concourse import mybir
    from concourse.tile import TileContext

    f32 = mybir.dt.float32
    bf16 = mybir.dt.bfloat16
    Alu = mybir.AluOpType
    Act = mybir.ActivationFunctionType

    nc = bacc.Bacc()
    # x, k-interleaved and quarter-major: one DMA slice brings every
    # contraction block for a 1024-node range (device node order is the
    # per-core rotated order; own nodes are quarters 0-1)
    xc_d = nc.declare_dram_parameter("xc", [128, KB * N], bf16, isOutput=False)
    Wh_d = nc.declare_dram_parameter("Wh", [128, KB * DA], bf16, isOutput=False)
    maskT_d = nc.declare_dram_parameter("maskT", [N, NOWN], bf16, isOutput=False)
    ones1_d = nc.declare_dram_parameter("ones1", [1, 128], f32, isOutput=False)
    out_d = nc.declare_dram_parameter("out", [NOWN, D], f32, isOutput=True)

    with TileContext(nc) as tc:
        with tc.tile_pool(name="const", bufs=1) as const_pool:
            xparts = [const_pool.tile([128, KB * NQ], bf16, name=f"xp{qt}")
                      for qt in range(4)]
            nc.sync.dma_start(xparts[0][:], xc_d[:, 0:KB * NQ])
            nc.sync.dma_start(xparts[1][:], xc_d[:, KB * NQ:2 * KB * NQ])
            Wh_sb = const_pool.tile([128, KB * DA], bf16)
            nc.sync.dma_start(Wh_sb[:], Wh_d[:])
            ones1 = const_pool.tile([1, 128], f32)
            nc.sync.dma_start(ones1[:], ones1_d[:])

            def xkb(k, b):
                """x block [128 k-rows, 128 nodes] for contraction k, node block b."""
                qt, bq = divmod(b, 8)
                off = k * NQ + bq * 128
                return xparts[qt][:, off:off + 128]

            haug = const_pool.tile([128, JB * (D + 1)], bf16)   # [h | 1] per block
            e1all = const_pool.tile([128, JB], f32)             # exp(s2) per block
            gall = const_pool.tile([128, JB], f32)              # exp(0.2 s2) per block
            sT_own = const_pool.tile([1, NOWN], f32)            # s1 row, own cols
            E8b = const_pool.tile([128, NOWN], bf16)            # exp(-0.8 s1) bcast

            # ones columns for the row-sum ride-along, one strided memset
            haug3 = haug.rearrange("p (b c) -> p b c", c=D + 1)
            nc.vector.memset(haug3[:, :, D:D + 1], 1.0)

            with (
                tc.tile_pool(name="stream", bufs=4) as stream,
                tc.tile_pool(name="tail", bufs=1) as tail_pool,
            ):
                def emit_h_block(b, pool):
                    ph = pool.tile([128, D + 1], f32, tag="ph")
                    for k in range(KB):
                        nc.tensor.matmul(
                            ph[:],
                            lhsT=xkb(k, b),
                            rhs=Wh_sb[:, k * DA:k * DA + D + 1],
                            start=(k == 0), stop=(k == KB - 1))
                    nc.scalar.activation(haug[:, b * (D + 1):b * (D + 1) + D],
                                         ph[:, 0:D], Act.Copy)
                    nc.scalar.activation(e1all[:, b:b + 1], ph[:, D:D + 1],
                                         Act.Exp)
                    nc.scalar.activation(gall[:, b:b + 1], ph[:, D:D + 1],
                                         Act.Exp, scale=0.2)

                with tc.tile_pool(name="pre", bufs=2, space="PSUM") as pre:
                    def emit_pst(c4):
                        # s1 for own cols c4*512.. (own = quarters 0/1)
                        pst = pre.tile([1, 512], f32, tag="pst")
                        qt, half = divmod(c4, 2)
                        for k in range(KB):
                            nc.tensor.matmul(
                                pst[:],
                                lhsT=Wh_sb[:, k * DA + D + 1:k * DA + D + 2],
                                rhs=xparts[qt][:, k * NQ + half * 512:
                                               k * NQ + (half + 1) * 512],
                                start=(k == 0), stop=(k == KB - 1))
                        nc.vector.tensor_copy(
                            sT_own[:, c4 * 512:(c4 + 1) * 512], pst[:])

                    def emit_e8b(t):
                        # broadcast s1 over partitions (one rank-1 matmul),
                        # then E8 = exp(-0.8 s1)
                        pb = pre.tile([128, 512], f32, tag="pb")
                        nc.tensor.matmul(
                            pb[:], lhsT=ones1[:],
                            rhs=sT_own[0:1, t * 512:(t + 1) * 512],
                            start=True, stop=True)
                        nc.scalar.activation(E8b[:, t * 512:(t + 1) * 512],
                                             pb[:], Act.Exp, scale=-0.8)

                    for c4 in range(4):
                        emit_pst(c4)
                    for t in range(4):
                        emit_e8b(t)
                    for b in range(4):
                        emit_h_block(b, pre)

                with (
                    tc.tile_pool(name="acc", bufs=1, space="PSUM") as acc_pool,
                    tc.tile_pool(name="hpsum", bufs=2, space="PSUM") as hpsum,
                    tc.tile_pool(name="mask8", bufs=3) as mask_pool,
                ):
                    # 5 tiles x 3 slices + 1 tile x 1 slice = 16 accumulators
                    acc = [acc_pool.tile([128, 3 * (D + 1)], f32,
                                         name=f"acc{t}") for t in range(5)]
                    acc.append(acc_pool.tile([128, D + 1], f32, name="acc5"))

                    def emit_attn_block(jb):
                        # q1 = max(E8_i * g_j, e1_j) in one two-scalar TS
                        q1 = stream.tile([128, NOWN], bf16, tag="q1")
                        nc.vector.tensor_scalar(q1[:], in0=E8b[:],
                                                scalar1=gall[:, jb:jb + 1],
                                                scalar2=e1all[:, jb:jb + 1],
                                                op0=Alu.mult, op1=Alu.max)
                        pm = stream.tile([128, NOWN], bf16, tag="pm")
                        s = jb % 4
                        nc.vector.tensor_tensor(
                            pm[:], q1[:],
                            masks[jb // 4][:, s * NOWN:(s + 1) * NOWN],
                            op=Alu.mult)

                        for ib in _MM_ORDER:
                            t8, s8 = divmod(ib, 3)
                            last = (s8 == 2) or (t8 == 5)
                            nc.tensor.matmul(
                                acc[t8][:, s8 * (D + 1):(s8 + 1) * (D + 1)],
                                lhsT=pm[:, ib * 128:(ib + 1) * 128],
                                rhs=haug[:, jb * (D + 1):(jb + 1) * (D + 1)],
                                start=(jb == 0 and s8 == 0),
                                stop=(jb == JB - 1 and last),
                                skip_group_check=True)

                    masks = {}

                    def issue_oct(oc):
                        mt = mask_pool.tile([128, 4 * NOWN], bf16, tag="m8")
                        m3 = mt.rearrange("p (s i) -> p s i", i=NOWN)
                        src3 = maskT_d[oc * 512:(oc + 1) * 512, :].rearrange(
                            "(s p) i -> p s i", p=128)
                        nc.sync.dma_start(m3[:], src3[:])
                        masks[oc] = mt

                    issue_oct(0)
                    issue_oct(1)
                    for step in range(4, JB + LAG):
                        if step == 6:
                            nc.sync.dma_start(xparts[2][:],
                                              xc_d[:, 2 * KB * NQ:3 * KB * NQ])
                        if step == 12:
                            nc.sync.dma_start(xparts[3][:],
                                              xc_d[:, 3 * KB * NQ:4 * KB * NQ])
                        if step in (8, 13, 17, 21, 25, 29):
                            issue_oct({8: 2, 13: 3, 17: 4, 21: 5,
                                       25: 6, 29: 7}[step])
                        if step < JB:
                            emit_h_block(step, hpsum)
                        if step >= LAG:
                            emit_attn_block(step - LAG)

                    # ---- tail: normalize + elu + store ----
                    osb = tail_pool.tile([128, NOWN], f32, tag="osb")
                    rinv = tail_pool.tile([128, IB], f32, tag="rinv")
                    for ib in range(IB):
                        t8, s = divmod(ib, 3)
                        nc.vector.reciprocal(
                            rinv[:, ib:ib + 1],
                            acc[t8][:, s * (D + 1) + D:s * (D + 1) + D + 1])
                    for ib in range(IB):
                        t8, s = divmod(ib, 3)
                        nc.scalar.activation(
                            osb[:, ib * 128:(ib + 1) * 128],
                            acc[t8][:, s * (D + 1):s * (D + 1) + D],
                            Act.Copy, scale=rinv[:, ib:ib + 1])
                    # elu(x) = (relu(x) - 1) + exp(min(x, 0))
                    zmin = tail_pool.tile([128, NOWN], f32, tag="zmin")
                    nc.vector.tensor_scalar(zmin[:], in0=osb[:], scalar1=0.0,
                                            scalar2=None, op0=Alu.min)
                    ez = tail_pool.tile([128, NOWN], f32, tag="ez")
                    nc.scalar.activation(ez[:], zmin[:], Act.Exp)
                    rm1 = tail_pool.tile([128, NOWN], f32, tag="rm1")
                    nc.vector.tensor_scalar(rm1[:], in0=osb[:], scalar1=0.0,
                                            scalar2=-1.0, op0=Alu.max,
                                            op1=Alu.add)
                    oo = tail_pool.tile([128, NOWN], f32, tag="oo")
                    nc.vector.tensor_tensor(oo[:], ez[:], rm1[:], op=Alu.add)
                    out3 = out_d.rearrange("(b p) d -> p b d", p=128)
                    oo3 = oo.rearrange("p (b d) -> p b d", d=D)
                    nc.sync.dma_start(out3[:], oo3[:])

    nc.compile()
    return nc


def _host_prep(x, adj, W, a):
    import ml_dtypes
    x = np.asarray(x, np.float32)
    adj = np.asarray(adj)
    W = np.asarray(W, np.float32)
    a = np.asarray(a, np.float32)

    xT = x.T.astype(ml_dtypes.bfloat16)                                # [F, N]
    adjT = adj.T.astype(ml_dtypes.bfloat16)                            # [j, i]
    ones1 = np.ones((1, 128), np.float32)

    in_maps = []
    for c in range(M):
        hd, q = divmod(c, 2)
        Wh = W[:, hd * D:(hd + 1) * D]                                  # [F, D]
        wa1 = Wh @ a[:D, 0]
        wa2 = Wh @ a[D:, 0]
        # [W_h | wa2 | wa1]: col D = s2 weights, col D+1 = s1 weights;
        # k-interleaved to load with one DMA
        Whc = np.concatenate([Wh, wa2[:, None], wa1[:, None]], axis=1)
        Whc = np.ascontiguousarray(
            Whc.astype(ml_dtypes.bfloat16).reshape(KB, 128, DA)
            .transpose(1, 0, 2).reshape(128, KB * DA))
        # rotate the node axis so own nodes are first, then k/quarter-interleave
        xrot = np.concatenate([xT[:, q * NOWN:], xT[:, :q * NOWN]], axis=1)
        xc = np.ascontiguousarray(
            xrot.reshape(KB, 128, 4, NQ).transpose(2, 1, 0, 3).reshape(4, 128, -1)
            .transpose(1, 0, 2).reshape(128, -1))
        adjrot = np.concatenate([adjT[q * NOWN:], adjT[:q * NOWN]], axis=0)
        in_maps.append({
            "xc": xc,
            "Wh": Whc,
            "maskT": np.ascontiguousarray(adjrot[:, q * NOWN:(q + 1) * NOWN]),
            "ones1": ones1,
        })
    return in_maps


def kernel(x, adj, W, a):
    from concourse.bass_utils import run_bass_kernel_spmd

    if "nc" not in _CACHE:
        _CACHE["nc"] = _build_nc()
    nc = _CACHE["nc"]

    in_maps = _host_prep(x, adj, W, a)
    res = run_bass_kernel_spmd(nc, in_maps, list(range(M)))
    out = np.empty((N, H * D), np.float32)
    for c in range(M):
        hd, q = divmod(c, 2)
        out[q * NOWN:(q + 1) * NOWN, hd * D:(hd + 1) * D] = np.asarray(
            res.results[c]["out"], np.float32)
    return out


if __name__ == "__main__":
    nc = _build_nc()
    print("built ok")


# revision 48
# speedup vs baseline: 1.2660x; 1.2660x over previous
"""HGATConv (4-head graph attention, N=4096, F=512) on 8 Trainium2 NeuronCores.

Sharding: (head, node-half) grid — core c handles head c//2 and output rows
q*2048..(q+1)*2048 (q = c%2). Every core computes its head's h = x @ W_h for
ALL nodes locally (bf16 matmuls) — no collective at all. The host rotates
the node axis per core so each core's own nodes come first (attention sums
are j-order invariant); that makes the per-core x layout identical SPMD-wise.

Attention math per 128-node j-block (tiles are [128 j, 2048 i]): all scores
are divided by exp(s1_i), which is constant per softmax row and cancels in
the normalization:
  p'' = exp(leakyrelu(s1_i + s2_j)) / exp(s1_i)
      = max(exp(-0.8 s1_i + 0.2 s2_j), exp(s2_j))
      = max(E8_i * g_j, e1_j)          (separable rank-1 first branch!)
with E8_i = exp(-0.8 s1_i) a jb-invariant broadcast tile, g_j = exp(0.2 s2_j)
and e1_j = exp(s2_j) per-partition scalars. So the whole score+leakyrelu is
ONE 4x-rate two-scalar tensor_scalar per block — no wide ACT exp at all; the
only wide 2x op left is the adjacency-mask multiply.
Row sums ride along as a ones-column appended to h (129-wide matmuls).

Mask DMAs issue from the (otherwise idle) GpSimd queue so the Sync queue's
serial dma_start issues don't gate the stream; x arrives in four
k-interleaved quarter DMAs so compute starts after the first MiB.
"""

import sys
import numpy as np

if "/opt/trn_rl_repo" not in sys.path:
    sys.path.insert(0, "/opt/trn_rl_repo")

H, D = 4, 128          # heads, head dim
N, F = 4096, 512       # nodes, features
M = 8                  # cores
NOWN = 1024 * 2        # 2048 own output rows per core
JB = N // 128          # 32 j blocks
IB = NOWN // 128       # 16 own-row blocks
KB = F // 128          # 4 contraction blocks
DA = D + 2             # head W columns + wa2 + wa1
NQ = N // 4            # nodes per x-quarter DMA
LAG = 4                # h-compute blocks emitted ahead of attention blocks

_CACHE = {}

# attention-matmul emission order: s=0 slices first (their start=True clears
# the bank), consecutive matmuls on different PSUM banks
_MM_ORDER = [0, 3, 6, 9, 12, 15, 1, 4, 7, 10, 13, 2, 5, 8, 11, 14]


def _build_nc():
    import concourse.bacc as bacc
    from concourse import mybir
    from concourse.tile import TileContext

    f32 = mybir.dt.float32
    bf16 = mybir.dt.bfloat16
    Alu = mybir.AluOpType
    Act = mybir.ActivationFunctionType

    nc = bacc.Bacc()
    # x, k-interleaved and quarter-major: one DMA slice brings every
    # contraction block for a 1024-node range (device node order is the
    # per-core rotated order; own nodes are quarters 0-1)
    xc_d = nc.declare_dram_parameter("xc", [128, KB * N], bf16, isOutput=False)
    Wh_d = nc.declare_dram_parameter("Wh", [128, KB * DA], bf16, isOutput=False)
    maskT_d = nc.declare_dram_parameter("maskT", [N, NOWN], bf16, isOutput=False)
    # wa1 replicated across 128 columns: E8b comes from one matmul layer
    wa1r_d = nc.declare_dram_parameter("wa1r", [128, KB * 128], bf16,
                                       isOutput=False)
    out_d = nc.declare_dram_parameter("out", [NOWN, D], f32, isOutput=True)

    with TileContext(nc) as tc:
        with tc.tile_pool(name="const", bufs=1) as const_pool:
            xparts = [const_pool.tile([128, KB * NQ], bf16, name=f"xp{qt}")
                      for qt in range(4)]
            nc.sync.dma_start(xparts[0][:], xc_d[:, 0:KB * NQ])
            nc.sync.dma_start(xparts[1][:], xc_d[:, KB * NQ:2 * KB * NQ])
            Wh_sb = const_pool.tile([128, KB * DA], bf16)
            nc.sync.dma_start(Wh_sb[:], Wh_d[:])
            wa1r = const_pool.tile([128, KB * 128], bf16)
            nc.sync.dma_start(wa1r[:], wa1r_d[:])

            def xkb(k, b):
                """x block [128 k-rows, 128 nodes] for contraction k, node block b."""
                qt, bq = divmod(b, 8)
                off = k * NQ + bq * 128
                return xparts[qt][:, off:off + 128]

            haug = const_pool.tile([128, JB * (D + 1)], bf16)   # [h | 1] per block
            e1all = const_pool.tile([128, JB], f32)             # exp(s2) per block
            gall = const_pool.tile([128, JB], f32)              # exp(0.2 s2) per block
            E8b = const_pool.tile([128, NOWN], bf16)            # exp(-0.8 s1) bcast

            # ones columns for the row-sum ride-along, one strided memset
            haug3 = haug.rearrange("p (b c) -> p b c", c=D + 1)
            nc.vector.memset(haug3[:, :, D:D + 1], 1.0)

            with (
                tc.tile_pool(name="stream", bufs=4) as stream,
                tc.tile_pool(name="tail", bufs=1) as tail_pool,
            ):
                def emit_h_block(b, pool):
                    ph = pool.tile([128, D + 1], f32, tag="ph")
                    for k in range(KB):
                        nc.tensor.matmul(
                            ph[:],
                            lhsT=xkb(k, b),
                            rhs=Wh_sb[:, k * DA:k * DA + D + 1],
                            start=(k == 0), stop=(k == KB - 1))
                    nc.scalar.activation(haug[:, b * (D + 1):b * (D + 1) + D],
                                         ph[:, 0:D], Act.Copy)
                    nc.scalar.activation(e1all[:, b:b + 1], ph[:, D:D + 1],
                                         Act.Exp)
                    nc.scalar.activation(gall[:, b:b + 1], ph[:, D:D + 1],
                                         Act.Exp, scale=0.2)

                with tc.tile_pool(name="pre", bufs=2, space="PSUM") as pre:
                    def emit_e8b(c4):
                        # pb[p, i] = s1_i for own cols c4*512.. directly from
                        # x via the replicated-wa1 stationary, then exp
                        pb = pre.tile([128, 512], f32, tag="pb")
                        qt, half = divmod(c4, 2)
                        for k in range(KB):
                            nc.tensor.matmul(
                                pb[:],
                                lhsT=wa1r[:, k * 128:(k + 1) * 128],
                                rhs=xparts[qt][:, k * NQ + half * 512:
                                               k * NQ + (half + 1) * 512],
                                start=(k == 0), stop=(k == KB - 1))
                        nc.scalar.activation(E8b[:, c4 * 512:(c4 + 1) * 512],
                                             pb[:], Act.Exp, scale=-0.8)

                    for c4 in range(4):
                        emit_e8b(c4)
                    for b in range(4):
                        emit_h_block(b, pre)

                with (
                    tc.tile_pool(name="acc", bufs=1, space="PSUM") as acc_pool,
                    tc.tile_pool(name="hpsum", bufs=2, space="PSUM") as hpsum,
                ):
                    # 5 tiles x 3 slices + 1 tile x 1 slice = 16 accumulators
                    acc = [acc_pool.tile([128, 3 * (D + 1)], f32,
                                         name=f"acc{t}") for t in range(5)]
                    acc.append(acc_pool.tile([128, D + 1], f32, name="acc5"))

                    def emit_attn_block(jb):
                        mask = stream.tile([128, NOWN], bf16, tag="mask")
                        nc.sync.dma_start(mask[:],
                                          maskT_d[jb * 128:(jb + 1) * 128, :])
                        # q1 = max(E8_i * g_j, e1_j) in one two-scalar TS
                        q1 = stream.tile([128, NOWN], bf16, tag="q1")
                        nc.vector.tensor_scalar(q1[:], in0=E8b[:],
                                                scalar1=gall[:, jb:jb + 1],
                                                scalar2=e1all[:, jb:jb + 1],
                                                op0=Alu.mult, op1=Alu.max)
                        pm = stream.tile([128, NOWN], bf16, tag="pm")
                        nc.vector.tensor_tensor(pm[:], q1[:], mask[:],
                                                op=Alu.mult)

                        for ib in _MM_ORDER:
                            t8, s8 = divmod(ib, 3)
                            last = (s8 == 2) or (t8 == 5)
                            nc.tensor.matmul(
                                acc[t8][:, s8 * (D + 1):(s8 + 1) * (D + 1)],
                                lhsT=pm[:, ib * 128:(ib + 1) * 128],
                                rhs=haug[:, jb * (D + 1):(jb + 1) * (D + 1)],
                                start=(jb == 0 and s8 == 0),
                                stop=(jb == JB - 1 and last),
                                skip_group_check=True)

                    for step in range(4, JB + LAG):
                        if step == 6:
                            nc.sync.dma_start(xparts[2][:],
                                              xc_d[:, 2 * KB * NQ:3 * KB * NQ])
                        if step == 12:
                            nc.sync.dma_start(xparts[3][:],
                                              xc_d[:, 3 * KB * NQ:4 * KB * NQ])
                        if step < JB:
                            emit_h_block(step, hpsum)
                        if step >= LAG:
                            emit_attn_block(step - LAG)

                    # ---- tail: normalize + elu + store ----
                    osb = tail_pool.tile([128, NOWN], f32, tag="osb")
                    rinv = tail_pool.tile([128, IB], f32, tag="rinv")
                    for ib in range(IB):
                        t8, s = divmod(ib, 3)
                        nc.vector.reciprocal(
                            rinv[:, ib:ib + 1],
                            acc[t8][:, s * (D + 1) + D:s * (D + 1) + D + 1])
                    for ib in range(IB):
                        t8, s = divmod(ib, 3)
                        nc.scalar.activation(
                            osb[:, ib * 128:(ib + 1) * 128],
                            acc[t8][:, s * (D + 1):s * (D + 1) + D],
                            Act.Copy, scale=rinv[:, ib:ib + 1])
                    # elu(x) = (relu(x) - 1) + exp(min(x, 0))
                    zmin = tail_pool.tile([128, NOWN], f32, tag="zmin")
                    nc.vector.tensor_scalar(zmin[:], in0=osb[:], scalar1=0.0,
                                            scalar2=None, op0=Alu.min)
                    ez = tail_pool.tile([128, NOWN], f32, tag="ez")
                    nc.scalar.activation(ez[:], zmin[:], Act.Exp)
                    rm1 = tail_pool.tile([128, NOWN], f32, tag="rm1")
                    nc.vector.tensor_scalar(rm1[:], in0=osb[:], scalar1=0.0,
                                            scalar2=-1.0, op0=Alu.max,
                                            op1=Alu.add)
                    oo = tail_pool.tile([128, NOWN], f32, tag="oo")
                    nc.vector.tensor_tensor(oo[:], ez[:], rm1[:], op=Alu.add)
                    out3 = out_d.rearrange("(b p) d -> p b d", p=128)
                    oo3 = oo.rearrange("p (b d) -> p b d", d=D)
                    nc.sync.dma_start(out3[:], oo3[:])

    nc.compile()
    return nc


def _host_prep(x, adj, W, a):
    import ml_dtypes
    x = np.asarray(x, np.float32)
    adj = np.asarray(adj)
    W = np.asarray(W, np.float32)
    a = np.asarray(a, np.float32)

    xT = x.T.astype(ml_dtypes.bfloat16)                                # [F, N]
    adjT = adj.T.astype(ml_dtypes.bfloat16)                            # [j, i]

    in_maps = []
    for c in range(M):
        hd, q = divmod(c, 2)
        Wh = W[:, hd * D:(hd + 1) * D]                                  # [F, D]
        wa1 = Wh @ a[:D, 0]
        wa2 = Wh @ a[D:, 0]
        # [W_h | wa2 | wa1]: col D = s2 weights, col D+1 = s1 weights;
        # k-interleaved to load with one DMA
        Whc = np.concatenate([Wh, wa2[:, None], wa1[:, None]], axis=1)
        Whc = np.ascontiguousarray(
            Whc.astype(ml_dtypes.bfloat16).reshape(KB, 128, DA)
            .transpose(1, 0, 2).reshape(128, KB * DA))
        # wa1 replicated across 128 stationary columns, k-interleaved
        wa1r = np.ascontiguousarray(
            np.broadcast_to(
                wa1.astype(ml_dtypes.bfloat16).reshape(KB, 128, 1),
                (KB, 128, 128)).transpose(1, 0, 2).reshape(128, KB * 128))
        # rotate the node axis so own nodes are first, then k/quarter-interleave
        xrot = np.concatenate([xT[:, q * NOWN:], xT[:, :q * NOWN]], axis=1)
        xc = np.ascontiguousarray(
            xrot.reshape(KB, 128, 4, NQ).transpose(2, 1, 0, 3).reshape(4, 128, -1)
            .transpose(1, 0, 2).reshape(128, -1))
        adjrot = np.concatenate([adjT[q * NOWN:], adjT[:q * NOWN]], axis=0)
        in_maps.append({
            "xc": xc,
            "Wh": Whc,
            "maskT": np.ascontiguousarray(adjrot[:, q * NOWN:(q + 1) * NOWN]),
            "wa1r": wa1r,
        })
    return in_maps


def kernel(x, adj, W, a):
    from concourse.bass_utils import run_bass_kernel_spmd

    if "nc" not in _CACHE:
        _CACHE["nc"] = _build_nc()
    nc = _CACHE["nc"]

    in_maps = _host_prep(x, adj, W, a)
    res = run_bass_kernel_spmd(nc, in_maps, list(range(M)))
    out = np.empty((N, H * D), np.float32)
    for c in range(M):
        hd, q = divmod(c, 2)
        out[q * NOWN:(q + 1) * NOWN, hd * D:(hd + 1) * D] = np.asarray(
            res.results[c]["out"], np.float32)
    return out


if __name__ == "__main__":
    nc = _build_nc()
    print("built ok")


# revision 51
# speedup vs baseline: 1.3186x; 1.0415x over previous
"""HGATConv (4-head graph attention, N=4096, F=512) on 8 Trainium2 NeuronCores.

Sharding: (head, node-half) grid — core c handles head c//2 and output rows
q*2048..(q+1)*2048 (q = c%2). Every core computes its head's h = x @ W_h for
ALL nodes locally (bf16 matmuls) — no collective at all. The host rotates
the node axis per core so each core's own nodes come first (attention sums
are j-order invariant); that makes the per-core x layout identical SPMD-wise.

Attention math per 128-node j-block (tiles are [128 j, 2048 i]): all scores
are divided by exp(s1_i), which is constant per softmax row and cancels in
the normalization:
  p'' = exp(leakyrelu(s1_i + s2_j)) / exp(s1_i)
      = max(exp(-0.8 s1_i + 0.2 s2_j), exp(s2_j))
      = max(E8_i * g_j, e1_j)          (separable rank-1 first branch!)
with E8_i = exp(-0.8 s1_i) a jb-invariant broadcast tile, g_j = exp(0.2 s2_j)
and e1_j = exp(s2_j) per-partition scalars. So the whole score+leakyrelu is
ONE 4x-rate two-scalar tensor_scalar per block — no wide ACT exp at all; the
only wide 2x op left is the adjacency-mask multiply.
Row sums ride along as a ones-column appended to h (129-wide matmuls).

Mask DMAs issue from the (otherwise idle) GpSimd queue so the Sync queue's
serial dma_start issues don't gate the stream; x arrives in four
k-interleaved quarter DMAs so compute starts after the first MiB.
"""

import sys
import numpy as np

if "/opt/trn_rl_repo" not in sys.path:
    sys.path.insert(0, "/opt/trn_rl_repo")

H, D = 4, 128          # heads, head dim
N, F = 4096, 512       # nodes, features
M = 8                  # cores
NOWN = 1024 * 2        # 2048 own output rows per core
JB = N // 128          # 32 j blocks
IB = NOWN // 128       # 16 own-row blocks
KB = F // 128          # 4 contraction blocks
DA = D + 2             # head W columns + wa2 + wa1
NQ = N // 4            # nodes per x-quarter DMA
LAG = 4                # h-compute blocks emitted ahead of attention blocks

_CACHE = {}

# attention-matmul emission order: s=0 slices first (their start=True clears
# the bank), consecutive matmuls on different PSUM banks
_MM_ORDER = [0, 3, 6, 9, 12, 15, 1, 4, 7, 10, 13, 2, 5, 8, 11, 14]


def _build_nc():
    import concourse.bacc as bacc
    from concourse import mybir
    from concourse.tile import TileContext

    f32 = mybir.dt.float32
    bf16 = mybir.dt.bfloat16
    Alu = mybir.AluOpType
    Act = mybir.ActivationFunctionType

    nc = bacc.Bacc()
    # x, k-interleaved and quarter-major: one DMA slice brings every
    # contraction block for a 1024-node range (device node order is the
    # per-core rotated order; own nodes are quarters 0-1)
    xc_d = nc.declare_dram_parameter("xc", [128, KB * N], bf16, isOutput=False)
    Wh_d = nc.declare_dram_parameter("Wh", [128, KB * DA], bf16, isOutput=False)
    maskT_d = nc.declare_dram_parameter("maskT", [N, NOWN], bf16, isOutput=False)
    # wa1 replicated across 128 columns: E8b comes from one matmul layer
    wa1r_d = nc.declare_dram_parameter("wa1r", [128, KB * 128], bf16,
                                       isOutput=False)
    out_d = nc.declare_dram_parameter("out", [NOWN, D], f32, isOutput=True)

    with TileContext(nc) as tc:
        with tc.tile_pool(name="const", bufs=1) as const_pool:
            Wh_sb = const_pool.tile([128, KB * DA], bf16)
            nc.sync.dma_start(Wh_sb[:], Wh_d[:])
            wa1r = const_pool.tile([128, KB * 128], bf16)
            nc.sync.dma_start(wa1r[:], wa1r_d[:])
            xparts = [const_pool.tile([128, KB * NQ], bf16, name=f"xp{qt}")
                      for qt in range(4)]
            nc.sync.dma_start(xparts[0][:], xc_d[:, 0:KB * NQ])
            nc.sync.dma_start(xparts[1][:], xc_d[:, KB * NQ:2 * KB * NQ])

            def xkb(k, b):
                """x block [128 k-rows, 128 nodes] for contraction k, node block b."""
                qt, bq = divmod(b, 8)
                off = k * NQ + bq * 128
                return xparts[qt][:, off:off + 128]

            haug = const_pool.tile([128, JB * (D + 1)], bf16)   # [h | 1] per block
            e1all = const_pool.tile([128, JB], f32)             # exp(s2) per block
            gall = const_pool.tile([128, JB], f32)              # exp(0.2 s2) per block
            E8b = const_pool.tile([128, NOWN], bf16)            # exp(-0.8 s1) bcast

            # ones columns for the row-sum ride-along, one strided memset
            haug3 = haug.rearrange("p (b c) -> p b c", c=D + 1)
            nc.vector.memset(haug3[:, :, D:D + 1], 1.0)

            with (
                tc.tile_pool(name="stream", bufs=5) as stream,
                tc.tile_pool(name="tail", bufs=1) as tail_pool,
            ):
                def emit_h_block(b, pool):
                    ph = pool.tile([128, D + 1], f32, tag="ph")
                    for k in range(KB):
                        nc.tensor.matmul(
                            ph[:],
                            lhsT=xkb(k, b),
                            rhs=Wh_sb[:, k * DA:k * DA + D + 1],
                            start=(k == 0), stop=(k == KB - 1))
                    nc.scalar.activation(haug[:, b * (D + 1):b * (D + 1) + D],
                                         ph[:, 0:D], Act.Copy)
                    nc.scalar.activation(e1all[:, b:b + 1], ph[:, D:D + 1],
                                         Act.Exp)
                    nc.scalar.activation(gall[:, b:b + 1], ph[:, D:D + 1],
                                         Act.Exp, scale=0.2)

                with tc.tile_pool(name="pre", bufs=2, space="PSUM") as pre:
                    def emit_e8b(c4):
                        # pb[p, i] = s1_i for own cols c4*512.. directly from
                        # x via the replicated-wa1 stationary, then exp
                        pb = pre.tile([128, 512], f32, tag="pb")
                        qt, half = divmod(c4, 2)
                        for k in range(KB):
                            nc.tensor.matmul(
                                pb[:],
                                lhsT=wa1r[:, k * 128:(k + 1) * 128],
                                rhs=xparts[qt][:, k * NQ + half * 512:
                                               k * NQ + (half + 1) * 512],
                                start=(k == 0), stop=(k == KB - 1))
                        nc.scalar.activation(E8b[:, c4 * 512:(c4 + 1) * 512],
                                             pb[:], Act.Exp, scale=-0.8)

                    for c4 in range(4):
                        emit_e8b(c4)
                    for b in range(4):
                        emit_h_block(b, pre)

                with (
                    tc.tile_pool(name="acc", bufs=1, space="PSUM") as acc_pool,
                    tc.tile_pool(name="hpsum", bufs=2, space="PSUM") as hpsum,
                ):
                    # 5 tiles x 3 slices + 1 tile x 1 slice = 16 accumulators
                    acc = [acc_pool.tile([128, 3 * (D + 1)], f32,
                                         name=f"acc{t}") for t in range(5)]
                    acc.append(acc_pool.tile([128, D + 1], f32, name="acc5"))

                    def emit_attn_block(jb):
                        mask = stream.tile([128, NOWN], bf16, tag="mask")
                        nc.sync.dma_start(mask[:],
                                          maskT_d[jb * 128:(jb + 1) * 128, :])
                        # q1 = max(E8_i * g_j, e1_j) in one two-scalar TS
                        q1 = stream.tile([128, NOWN], bf16, tag="q1")
                        nc.vector.tensor_scalar(q1[:], in0=E8b[:],
                                                scalar1=gall[:, jb:jb + 1],
                                                scalar2=e1all[:, jb:jb + 1],
                                                op0=Alu.mult, op1=Alu.max)
                        pm = stream.tile([128, NOWN], bf16, tag="pm")
                        nc.vector.tensor_tensor(pm[:], q1[:], mask[:],
                                                op=Alu.mult)

                        for ib in _MM_ORDER:
                            t8, s8 = divmod(ib, 3)
                            last = (s8 == 2) or (t8 == 5)
                            nc.tensor.matmul(
                                acc[t8][:, s8 * (D + 1):(s8 + 1) * (D + 1)],
                                lhsT=pm[:, ib * 128:(ib + 1) * 128],
                                rhs=haug[:, jb * (D + 1):(jb + 1) * (D + 1)],
                                start=(jb == 0 and s8 == 0),
                                stop=(jb == JB - 1 and last),
                                skip_group_check=True)

                    for step in range(4, JB + LAG):
                        if step == 6:
                            nc.sync.dma_start(xparts[2][:],
                                              xc_d[:, 2 * KB * NQ:3 * KB * NQ])
                        if step == 12:
                            nc.sync.dma_start(xparts[3][:],
                                              xc_d[:, 3 * KB * NQ:4 * KB * NQ])
                        if step < JB:
                            emit_h_block(step, hpsum)
                        if step >= LAG:
                            emit_attn_block(step - LAG)

                    # ---- tail: normalize + elu + store ----
                    osb = tail_pool.tile([128, NOWN], f32, tag="osb")
                    rinv = tail_pool.tile([128, IB], f32, tag="rinv")
                    for ib in range(IB):
                        t8, s = divmod(ib, 3)
                        nc.vector.reciprocal(
                            rinv[:, ib:ib + 1],
                            acc[t8][:, s * (D + 1) + D:s * (D + 1) + D + 1])
                    for ib in range(IB):
                        t8, s = divmod(ib, 3)
                        nc.scalar.activation(
                            osb[:, ib * 128:(ib + 1) * 128],
                            acc[t8][:, s * (D + 1):s * (D + 1) + D],
                            Act.Copy, scale=rinv[:, ib:ib + 1])
                    # elu(x) = (relu(x) - 1) + exp(min(x, 0)), two halves so
                    # ACT/DVE/DMA pipeline across them
                    zmin = tail_pool.tile([128, NOWN], f32, tag="zmin")
                    ez = tail_pool.tile([128, NOWN], f32, tag="ez")
                    rm1 = tail_pool.tile([128, NOWN], f32, tag="rm1")
                    oo = tail_pool.tile([128, NOWN], f32, tag="oo")
                    out3 = out_d.rearrange("(b p) d -> p b d", p=128)
                    oo3 = oo.rearrange("p (b d) -> p b d", d=D)
                    HN = NOWN // 2
                    for hf in range(2):
                        sl = slice(hf * HN, (hf + 1) * HN)
                        nc.vector.tensor_scalar(zmin[:, sl], in0=osb[:, sl],
                                                scalar1=0.0, scalar2=None,
                                                op0=Alu.min)
                        nc.scalar.activation(ez[:, sl], zmin[:, sl], Act.Exp)
                        nc.vector.tensor_scalar(rm1[:, sl], in0=osb[:, sl],
                                                scalar1=0.0, scalar2=-1.0,
                                                op0=Alu.max, op1=Alu.add)
                        nc.vector.tensor_tensor(oo[:, sl], ez[:, sl],
                                                rm1[:, sl], op=Alu.add)
                        bs = slice(hf * (IB // 2), (hf + 1) * (IB // 2))
                        nc.sync.dma_start(out3[:, bs, :], oo3[:, bs, :])

    nc.compile()
    return nc


def _host_prep(x, adj, W, a):
    import ml_dtypes
    x = np.asarray(x, np.float32)
    adj = np.asarray(adj)
    W = np.asarray(W, np.float32)
    a = np.asarray(a, np.float32)

    xT = x.T.astype(ml_dtypes.bfloat16)                                # [F, N]
    adjT = adj.T.astype(ml_dtypes.bfloat16)                            # [j, i]

    in_maps = []
    for c in range(M):
        hd, q = divmod(c, 2)
        Wh = W[:, hd * D:(hd + 1) * D]                                  # [F, D]
        wa1 = Wh @ a[:D, 0]
        wa2 = Wh @ a[D:, 0]
        # [W_h | wa2 | wa1]: col D = s2 weights, col D+1 = s1 weights;
        # k-interleaved to load with one DMA
        Whc = np.concatenate([Wh, wa2[:, None], wa1[:, None]], axis=1)
        Whc = np.ascontiguousarray(
            Whc.astype(ml_dtypes.bfloat16).reshape(KB, 128, DA)
            .transpose(1, 0, 2).reshape(128, KB * DA))
        # wa1 replicated across 128 stationary columns, k-interleaved
        wa1r = np.ascontiguousarray(
            np.broadcast_to(
                wa1.astype(ml_dtypes.bfloat16).reshape(KB, 128, 1),
                (KB, 128, 128)).transpose(1, 0, 2).reshape(128, KB * 128))
        # rotate the node axis so own nodes are first, then k/quarter-interleave
        xrot = np.concatenate([xT[:, q * NOWN:], xT[:, :q * NOWN]], axis=1)
        xc = np.ascontiguousarray(
            xrot.reshape(KB, 128, 4, NQ).transpose(2, 1, 0, 3).reshape(4, 128, -1)
            .transpose(1, 0, 2).reshape(128, -1))
        adjrot = np.concatenate([adjT[q * NOWN:], adjT[:q * NOWN]], axis=0)
        in_maps.append({
            "xc": xc,
            "Wh": Whc,
            "maskT": np.ascontiguousarray(adjrot[:, q * NOWN:(q + 1) * NOWN]),
            "wa1r": wa1r,
        })
    return in_maps


def kernel(x, adj, W, a):
    from concourse.bass_utils import run_bass_kernel_spmd

    if "nc" not in _CACHE:
        _CACHE["nc"] = _build_nc()
    nc = _CACHE["nc"]

    in_maps = _host_prep(x, adj, W, a)
    res = run_bass_kernel_spmd(nc, in_maps, list(range(M)))
    out = np.empty((N, H * D), np.float32)
    for c in range(M):
        hd, q = divmod(c, 2)
        out[q * NOWN:(q + 1) * NOWN, hd * D:(hd + 1) * D] = np.asarray(
            res.results[c]["out"], np.float32)
    return out


if __name__ == "__main__":
    nc = _build_nc()
    print("built ok")
